# revision 10
# baseline (speedup 1.0000x reference)
"""Trainium2 kernel for nn_GuidedMoEBasic (moe_routing).

Reference computation (B=8 docs, D=128 tokens, H=768, NE=7, E=2 experts):
  emotion = pooled @ We + be                                  [1024, 7]
  utter   = [pooled | emotion | speaker]                      [B, D, 776]
  pair(t,end) = [utter[t] | utter[end]]  for t <= end         [B, 8256, 1552]
  gating  = pair @ Wg + bg                                    [N, 2]
  expert_e = pair @ W1[e] @ W2[e] + b1[e] @ W2[e] + b2[e]     [E, N, 2]
  cause   = sum_e expert_e * gating[:, e]                     [N, 2]

Key algebra: every per-pair quantity is a SUM of a t-side and an end-side
linear function of single-token features, so nothing pair-shaped is ever
materialized. With
  x_e[t,c] = t-side expert features (+ all constant terms)
  y_e[n,c] = end-side expert features
  p_e[t]   = t-side gating (+ bg),  q_e[n] = end-side gating
each output channel c is a rank-6 product:
  cause(t,end,c) = S1_c[t] + sum_e x_e q_e + sum_e p_e y_e + S2_c[end]
                 = U_c^T V_c,  U_c = [x_0, x_1, p_0, p_1, S1_c, 1],
                               V_c = [q_0, q_1, y_0, y_1, 1, S2_c]

The 776/1552-dim contractions fold (host, weights only) into two [768, 8]
matrices. Per core (= one document) the device computes X^T via PE
transpose, two 8-row feature chains A/B (transposed orientation), the
elementwise S-rows, assembles U/V via a PE row-selection matmul (compute
engines need 32-aligned partition bases, so rows are gathered from
quadrant-anchored slots of a stack tile), then two [6x128]^T@[6x128]
matmuls -> dense [128, 2*128]. Host extracts the tril pairs.

Stack tile row map: 0:8 = chain A rows [x00,x10,x01,x11,q0,q1,q0,q1],
32:36 = [S1_0,S1_1,S2_0,S2_1], 64:72 = chain B rows
[p0,p1,p0,p1,y00,y10,y01,y11], 96:97 = ones.

All weights/constants ship in two packed DRAM tensors (wpack, consts) to
keep per-instruction semaphore-wait counts low (codegen limit).
"""

import numpy as np

B, D, H, NE = 8, 128, 768, 7
NEXP = 2
KC = H // 128

_CACHE = {}

# stack-row indices for [U0, U1, V0, V1] selection (6 rows each)
_SEL_ROWS = [
    [0, 1, 64, 65, 32, 96],
    [2, 3, 64, 65, 33, 96],
    [4, 5, 68, 69, 96, 34],
    [6, 7, 70, 71, 96, 35],
]

# consts column layout
_C_IDENT = 0  # [128, 0:128] identity
_C_SEL = 128  # [128, 128:152] selection matrices
_C_SCA = 152  # [2, 152:160]
_C_SCB = 160  # [2, 160:168]
_C_BE = 168  # [1, 168:175]
_C_RT = 175  # [8, 175:179]
_C_W = 179  # consts width


def _build_nc():
    import concourse.bacc as bacc
    import concourse.mybir as mybir
    import concourse.tile as tile

    f32 = mybir.dt.float32
    # Bacc (not raw Bass): its compile() splits multi-wait sync_info into
    # event semaphores / ldweights waits — raw Tile BIR violates the
    # one-wait-per-instruction codegen limit.
    nc = bacc.Bacc(None, target_bir_lowering=False)

    xb_d = nc.declare_dram_parameter("xb", [D, H], f32, isOutput=False)
    spko_d = nc.declare_dram_parameter("spko", [2, D], f32, isOutput=False)
    wpack_d = nc.declare_dram_parameter("wpack", [128, KC * 23], f32, isOutput=False)
    consts_d = nc.declare_dram_parameter("consts", [128, _C_W], f32, isOutput=False)
    emo_d = nc.declare_dram_parameter("emo", [D, NE], f32, isOutput=True)
    dense_d = nc.declare_dram_parameter("dense", [D, 2 * D], f32, isOutput=True)

    with tile.TileContext(nc) as tc:
        with (
            tc.tile_pool(name="const", bufs=1) as cpool,
            tc.tile_pool(name="work", bufs=1) as wpool,
            tc.tile_pool(name="ps", bufs=2, space="PSUM") as ps2,
            tc.tile_pool(name="psa", bufs=1, space="PSUM") as ps1,
        ):
            x = cpool.tile([D, H], f32)
            nc.sync.dma_start(x[:], xb_d[:])
            spko = cpool.tile([2, D], f32)
            nc.sync.dma_start(spko[:], spko_d[:])
            wp = cpool.tile([128, KC, 23], f32)
            nc.sync.dma_start(wp[:], wpack_d[:].rearrange("p (c j) -> p c j", j=23))
            ct = cpool.tile([128, _C_W], f32)
            nc.sync.dma_start(ct[:], consts_d[:])
            ident = ct[:, _C_IDENT : _C_IDENT + 128]
            sca = ct[0:2, _C_SCA : _C_SCA + 8]
            scb = ct[0:2, _C_SCB : _C_SCB + 8]
            berow = ct[0:1, _C_BE : _C_BE + NE]
            rt = ct[0:8, _C_RT : _C_RT + 4]
            ones = cpool.tile([1, D], f32)
            nc.vector.memset(ones[:], 1.0)

            # Priming PE ops: walrus codegen allows only ONE semaphore wait
            # per Matmult, so absorb each input-DMA dependency into its own
            # throwaway PE instruction; later matmuls then carry at most one
            # new wait thanks to Tile's per-engine vector clock.
            j0 = ps2.tile([128, D], f32, tag="big")
            nc.tensor.transpose(j0[:], ident, ident)
            j1 = ps2.tile([128, D], f32, tag="big")
            nc.tensor.matmul(j1[:], spko[:], spko[:], start=True, stop=True)
            j2 = ps1.tile([8, D], f32, tag="s")
            nc.tensor.matmul(j2[:], wp[:, 0, 0:8], ident, start=True, stop=True)

            # X^T chunks via PE transpose
            xt = cpool.tile([128, KC, D], f32)
            for c in range(KC):
                tp = ps2.tile([128, D], f32, tag="big")
                nc.tensor.transpose(tp[:], x[:, 128 * c : 128 * (c + 1)], ident)
                nc.vector.tensor_copy(xt[:, c, :], tp[:])

            # feature chains (transposed orientation): A/B [8, 128]
            gta = ps1.tile([8, D], f32, tag="gta")
            gtb = ps1.tile([8, D], f32, tag="gtb")
            for c in range(KC):
                nc.tensor.matmul(
                    gta[:], wp[:, c, 0:8], xt[:, c, :], start=(c == 0), stop=False
                )
            nc.tensor.matmul(gta[:], sca, spko[:], start=False, stop=True)
            for c in range(KC):
                nc.tensor.matmul(
                    gtb[:], wp[:, c, 8:16], xt[:, c, :], start=(c == 0), stop=False
                )
            nc.tensor.matmul(gtb[:], scb, spko[:], start=False, stop=True)

            # emotion head, natural orientation [128 tokens, 7]
            emo_ps = ps1.tile([D, NE], f32, tag="emo")
            for c in range(KC):
                nc.tensor.matmul(
                    emo_ps[:], xt[:, c, :], wp[:, c, 16:23], start=(c == 0), stop=False
                )
            nc.tensor.matmul(emo_ps[:], ones[:], berow, start=False, stop=True)
            emo_sb = wpool.tile([D, NE], f32, tag="emos")
            nc.vector.tensor_copy(emo_sb[:], emo_ps[:])
            nc.sync.dma_start(emo_d[:], emo_sb[:])

            # quadrant-anchored stack: A rows, B rows, ones, then S rows via PE
            stack = wpool.tile([D, D], f32, tag="stack")
            nc.vector.memset(stack[:], 0.0)
            nc.vector.tensor_copy(stack[0:8, :], gta[:])
            nc.vector.tensor_copy(stack[64:72, :], gtb[:])
            nc.vector.tensor_copy(stack[96:97, :], ones[:])
            prod = wpool.tile([8, D], f32, tag="prod")
            nc.vector.tensor_mul(prod[:], stack[0:8, :], gtb[:])
            s_ps = ps1.tile([4, D], f32, tag="s")
            nc.tensor.matmul(s_ps[:], rt, prod[:], start=True, stop=True)
            nc.vector.tensor_copy(stack[32:36, :], s_ps[:])

            # row-selection matmuls -> U/V factors in SBUF
            uv = []
            for i in range(4):
                u_ps = ps2.tile([6, D], f32, tag="uv")
                nc.tensor.matmul(
                    u_ps[:],
                    ct[:, _C_SEL + 6 * i : _C_SEL + 6 * i + 6],
                    stack[:],
                    start=True,
                    stop=True,
                )
                u_sb = wpool.tile([6, D], f32, tag=f"uv{i}")
                nc.vector.tensor_copy(u_sb[:], u_ps[:])
                uv.append(u_sb)

            dense_sb = wpool.tile([D, 2 * D], f32, tag="dense")
            for ch in range(2):
                dps = ps2.tile([D, D], f32, tag="big")
                nc.tensor.matmul(dps[:], uv[ch][:], uv[2 + ch][:], start=True, stop=True)
                nc.vector.tensor_copy(dense_sb[:, D * ch : D * (ch + 1)], dps[:])
            nc.sync.dma_start(dense_d[:], dense_sb[:])

    nc.compile()
    return nc


def _fold_weights(We, be, Wg, bg, W1, b1, W2, b2):
    """Fold everything into chain matrices wa/wb [768, 8] + K=2 constant
    blocks sca/scb [2, 8] (row 0 scales speaker, row 1 scales ones).

    utter = [pooled(768) | emo(7) | spk(1)], emo = pooled@We + be, so for a
    776-row block Ws: utter @ Ws = pooled @ (Ws[:768] + We@Ws[768:775])
    + spk * Ws[775] + be @ Ws[768:775].
    """

    def fold(Ws):  # [776, m] -> eff [768, m], s [m], c [m]
        return (
            Ws[:H] + We @ Ws[H : H + NE],
            Ws[H + NE],
            be @ Ws[H : H + NE],
        )

    Aeff, As, Ac, Beff, Bs, Bc = [], [], [], [], [], []
    for e in range(NEXP):
        t_eff, t_s, t_c = fold(W1[e][: H + NE + 1])
        e_eff, e_s, e_c = fold(W1[e][H + NE + 1 :])
        K_e = b1[e] @ W2[e] + b2[e]
        Aeff.append(t_eff @ W2[e])
        As.append(t_s @ W2[e])
        Ac.append(t_c @ W2[e] + K_e)  # all pair-constant expert terms on t side
        Beff.append(e_eff @ W2[e])
        Bs.append(e_s @ W2[e])
        Bc.append(e_c @ W2[e])
    Gt_eff, Gt_s, Gt_c = fold(Wg[: H + NE + 1])
    Ge_eff, Ge_s, Ge_c = fold(Wg[H + NE + 1 :])
    Gt_c = Gt_c + bg

    # A cols: [x00, x10, x01, x11, q0, q1, q0, q1]
    # B cols: [p0, p1, p0, p1, y00, y10, y01, y11]
    wa = np.stack(
        [Aeff[0][:, 0], Aeff[1][:, 0], Aeff[0][:, 1], Aeff[1][:, 1]]
        + [Ge_eff[:, 0], Ge_eff[:, 1]] * 2,
        axis=1,
    ).astype(np.float32)
    sca = np.stack(
        [
            [As[0][0], As[1][0], As[0][1], As[1][1]] + [Ge_s[0], Ge_s[1]] * 2,
            [Ac[0][0], Ac[1][0], Ac[0][1], Ac[1][1]] + [Ge_c[0], Ge_c[1]] * 2,
        ]
    ).astype(np.float32)
    wb = np.stack(
        [Gt_eff[:, 0], Gt_eff[:, 1]] * 2
        + [Beff[0][:, 0], Beff[1][:, 0], Beff[0][:, 1], Beff[1][:, 1]],
        axis=1,
    ).astype(np.float32)
    scb = np.stack(
        [
            [Gt_s[0], Gt_s[1]] * 2 + [Bs[0][0], Bs[1][0], Bs[0][1], Bs[1][1]],
            [Gt_c[0], Gt_c[1]] * 2 + [Bc[0][0], Bc[1][0], Bc[0][1], Bc[1][1]],
        ]
    ).astype(np.float32)
    return wa, sca, wb, scb


def _sel_matrix():
    sel = np.zeros((D, 24), np.float32)
    for i, rows in enumerate(_SEL_ROWS):
        for k, r in enumerate(rows):
            sel[r, 6 * i + k] = 1.0
    return sel


def _rt_matrix():
    rt = np.zeros((8, 4), np.float32)
    for j in range(4):
        rt[2 * j, j] = 1.0
        rt[2 * j + 1, j] = 1.0
    return rt


def _pack_inputs(inputs):
    pooled = np.ascontiguousarray(np.asarray(inputs["pooled_output"], np.float32))
    spk = np.asarray(inputs["speaker_ids"], np.float32)
    We = np.asarray(inputs["We"], np.float32)
    be = np.asarray(inputs["be"], np.float32)
    wa, sca, wb, scb = _fold_weights(
        We,
        be,
        np.asarray(inputs["Wg"], np.float32),
        np.asarray(inputs["bg"], np.float32),
        np.asarray(inputs["W1"], np.float32),
        np.asarray(inputs["b1"], np.float32),
        np.asarray(inputs["W2"], np.float32),
        np.asarray(inputs["b2"], np.float32),
    )

    wcat = np.hstack([wa, wb, We])  # [768, 23]
    wpack = np.ascontiguousarray(
        wcat.reshape(KC, 128, 23).transpose(1, 0, 2).reshape(128, KC * 23)
    )

    consts = np.zeros((128, _C_W), np.float32)
    consts[:, _C_IDENT : _C_IDENT + 128] = np.eye(D, dtype=np.float32)
    consts[:, _C_SEL : _C_SEL + 24] = _sel_matrix()
    consts[0:2, _C_SCA : _C_SCA + 8] = sca
    consts[0:2, _C_SCB : _C_SCB + 8] = scb
    consts[0:1, _C_BE : _C_BE + NE] = be[None, :]
    consts[0:8, _C_RT : _C_RT + 4] = _rt_matrix()

    shared = {"wpack": wpack, "consts": consts}
    in_maps = [
        {
            "xb": np.ascontiguousarray(pooled[b * D : (b + 1) * D]),
            "spko": np.stack([spk[b], np.ones(D, np.float32)]),
            **shared,
        }
        for b in range(B)
    ]
    return in_maps


def _run(inputs, trace=False, **spmd_kwargs):
    from concourse.bass_utils import run_bass_kernel_spmd

    in_maps = _pack_inputs(inputs)
    if "nc" not in _CACHE:
        _CACHE["nc"] = _build_nc()
    nc = _CACHE["nc"]
    res = run_bass_kernel_spmd(nc, in_maps, list(range(B)), trace=trace, **spmd_kwargs)

    emotion = np.concatenate([np.asarray(r["emo"]) for r in res.results], axis=0)
    e_idx, t_idx = np.tril_indices(D)
    cause = np.concatenate(
        [
            np.asarray(r["dense"]).reshape(D, 2, D)[t_idx, :, e_idx]
            for r in res.results
        ],
        axis=0,
    )
    return (emotion.astype(np.float32), cause.astype(np.float32)), res


def kernel(**inputs):
    outs, _ = _run(inputs, trace=False)
    return outs
